# revision 20
# baseline (speedup 1.0000x reference)
"""MoE feed-forward (top-1 routing, capacity 640, swiglu experts) on 8 trn2 cores.

Strategy (feature-parallel / H-shard):
  * Host: router matmul/softmax/argmax + capacity-slot assignment, gathers
    tokens per expert (identical token stream for every core).
  * Device (Bass/Tile, per core): every core processes ALL routed tokens of
    all 16 experts, but only a 1/8 slice of the FFN hidden dimension
    (384 of 3072 swiglu features per expert).  This removes the expert
    load-imbalance entirely - per-core work is exactly 1/8 of total
    regardless of routing.  Grouped GEMM  hT = W1s^T x  -> swiglu ->
    yT_partial = W2s^T g  in bf16 with fp32 accumulate; tokens stay in the
    moving dimension (exact counts, 8-aligned).  The 8 partial yT (each
    summing 384 of the 3072 hidden contributions) are added on the host.
  * Startup: the first x chunk + first W1 tile ride ONE prologue DMA
    (each dma_start costs ~1us serialized: descriptor-gen + doorbell);
    b1 biases ride inside every W1 tile (bitcast bf16 pairs); dummy
    matmuls on a zeroed tile warm the PE clock gate (HAM un-throttle)
    during the initial DMA wait.
  * DMA pacing: all input streams share the sync queue in consumption
    order (x block, 3 W1 tiles, W2 slice per expert); y partial stores
    alternate between the gpsimd and sync queues.
  * Host: sum the 8 partial yT, gate-weight, scatter back to token order;
    dense fallback FFN applied only to dropped tokens.
"""

import os
import sys

import numpy as np


def _ensure_concourse():
    try:
        import concourse.bass  # noqa: F401
    except Exception:
        for p in ("/opt/trn_rl_repo", "/root/.axon_site/_ro/trn_rl_repo"):
            if os.path.isdir(p) and p not in sys.path:
                sys.path.insert(0, p)
        import concourse.bass  # noqa: F401


# Problem constants (hardcoded per the task contract).
B, S, D, H, E = 4, 2048, 768, 3072, 16
N = B * S
C = 640  # capacity per expert (ceil(1.25 * N / E))
FALLBACK_W = 1.0
NCORES = 8
KD = D // 128  # 6 k-tiles for GEMM1 contraction
FB = (2 * H) // 128  # 48 feature blocks of GEMM1 output
FP = FB // 2  # 24 swiglu pairs total
FPS = FP // NCORES  # 3 swiglu pairs per core
KHS = H // 128 // NCORES  # 3 k-tiles of GEMM2 contraction per core
DT = D // 128  # 6 output d-tiles of GEMM2
W1W = 2 * KD * 128  # 1536 weight columns of one W1 tile
W1T = W1W + 4  # + 2 fp32 bias columns packed as 4 bf16
NWARM = 40  # dummy matmuls to pre-warm the PE clock gate

_NC_CACHE = {}  # Ls tuple -> compiled Bass program
_WCACHE = {}  # weight reorder cache
LAST = None  # BassKernelResults of the most recent run (for profiling)


def _chunks(L):
    """Split token count L into near-equal moving chunks <= 512 (8-aligned)."""
    nch = -(-L // 512)
    base = -(-(-(-L // nch)) // 8) * 8
    out, off = [], 0
    for _ in range(nch - 1):
        out.append((off, base))
        off += base
    out.append((off, L - off))
    return out


def _g1_chunks(L, first=False):
    """GEMM1 chunking: the first expert gets a 128-token head chunk that
    rides the prologue DMA; an n + (L-n) col matmul pair costs exactly what
    two L/2-col matmuls do, so the split is free."""
    if first and L > 128:
        return [(0, 128), (128, L - 128)]
    if L <= 512:
        return [(0, L)]
    return [(0, 128), (128, L - 128)]


def _build_nc(Ls):
    """Per-core Bass program: all E experts, Ls[e] (8-aligned) tokens each,
    FPS swiglu pairs (= KHS g-feature k-tiles) per expert."""
    import concourse.bacc as bacc
    import concourse.mybir as mybir
    import concourse.tile as tile
    from contextlib import ExitStack

    f32 = mybir.dt.float32
    bf16 = mybir.dt.bfloat16
    AF = mybir.ActivationFunctionType
    ALU = mybir.AluOpType

    tot = sum(Ls)
    pro_x = KD * _g1_chunks(Ls[0], True)[0][1]  # x columns in prologue pack

    nc = bacc.Bacc("TRN2", target_bir_lowering=False)
    # pro packs expert-0's first x token-chunk + its first W1 tile (with
    # biases).  xt is chunk-major: per expert, per remaining token chunk,
    # a [128, KD*cn] block.  w1r holds FPS tiles per expert (this core's
    # feature slice); w2t holds this core's KHS h k-tiles per expert.
    pro = nc.dram_tensor("pro", [128, pro_x + W1T], bf16, kind="ExternalInput")
    xt = nc.dram_tensor("xt", [128, KD * tot - pro_x], bf16, kind="ExternalInput")
    w1r = nc.dram_tensor("w1r", [E, FPS, 128, W1T], bf16, kind="ExternalInput")
    w2t = nc.dram_tensor("w2t", [E, 128, KHS * D], bf16, kind="ExternalInput")
    y = nc.dram_tensor("y", [D, tot], bf16, kind="ExternalOutput")

    with tile.TileContext(nc) as tc, ExitStack() as ctx:
        prop = ctx.enter_context(tc.tile_pool(name="prop", bufs=1))
        xp = ctx.enter_context(tc.tile_pool(name="xp", bufs=3))
        w2p = ctx.enter_context(tc.tile_pool(name="w2p", bufs=3))
        gp = ctx.enter_context(tc.tile_pool(name="gp", bufs=3))
        w1p = ctx.enter_context(tc.tile_pool(name="w1p", bufs=8))
        sap = ctx.enter_context(tc.tile_pool(name="sap", bufs=3))
        cst = ctx.enter_context(tc.tile_pool(name="cst", bufs=1))
        yp = ctx.enter_context(tc.tile_pool(name="yp", bufs=4))
        p1 = ctx.enter_context(tc.tile_pool(name="p1", bufs=3, space="PSUM"))
        p2 = ctx.enter_context(tc.tile_pool(name="p2", bufs=2, space="PSUM"))

        # Prologue pack: first load on the sync queue.
        pro_sb = prop.tile([128, pro_x + W1T], bf16, tag="pro")
        nc.sync.dma_start(pro_sb[:], pro[:])

        # Pre-warm the PE clock gate (HAM) with dummy matmuls on a zeroed
        # tile while the first DMAs are in flight.
        zt = cst.tile([128, 128], bf16, tag="zt")
        nc.vector.memset(zt[:], 0.0)
        pz = p2.tile([128, 128], f32, tag="p2")
        for i in range(NWARM):
            nc.tensor.matmul(
                pz[:], lhsT=zt[:], rhs=zt[:], start=(i == 0), stop=(i == NWARM - 1)
            )

        xoff = 0  # running column offset into xt
        yoff = 0  # running token offset into y
        ndrain = 0  # alternation counter for drain engine / DMA queue
        for e in range(E):
            L = Ls[e]
            tiles = _g1_chunks(L, e == 0)
            xaps = []
            rest = tiles
            if e == 0:
                xaps.append(pro_sb[:, :pro_x])
                rest = tiles[1:]
            if rest:
                xw = sum(KD * cn for _, cn in rest)
                xsb = xp.tile([128, xw], bf16, tag="x")
                boff = 0
                for _, cn in rest:
                    xaps.append(xsb[:, boff : boff + KD * cn])
                    nc.sync.dma_start(
                        xsb[:, boff : boff + KD * cn], xt[:, xoff : xoff + KD * cn]
                    )
                    boff += KD * cn
                    xoff += KD * cn

            gt = gp.tile([128, KHS * L], bf16, tag="g")
            w2sb = w2p.tile([128, KHS * D], bf16, tag="w2")

            # GEMM1 + swiglu: hT tiles [feat 128, tok chunk]
            for fp in range(FPS):
                if e == 0 and fp == 0:
                    w1t = pro_sb[:, pro_x:]
                else:
                    w1t = w1p.tile([128, W1T], bf16, tag="w1")
                    nc.sync.dma_start(w1t[:], w1r[e, fp, :, :])
                    w1t = w1t[:]
                if fp > 0:
                    # trickle this expert's W2 slice behind its W1 tiles
                    j = fp - 1
                    w = (KHS * D) // (FPS - 1)
                    nc.sync.dma_start(
                        w2sb[:, j * w : (j + 1) * w], w2t[e, :, j * w : (j + 1) * w]
                    )
                w1a = w1t[:, : KD * 128]
                w1b = w1t[:, KD * 128 : W1W]
                bia = w1t[:, W1W : W1W + 2].bitcast(f32)
                bib = w1t[:, W1W + 2 : W1W + 4].bitcast(f32)
                for ci, (toff, tn) in enumerate(tiles):
                    xb = xaps[ci]
                    pa = p1.tile([128, tn], f32, tag="pa")
                    pb = p1.tile([128, tn], f32, tag="pb")
                    for k in range(KD):
                        nc.tensor.matmul(
                            pa[:],
                            lhsT=w1a[:, k * 128 : (k + 1) * 128],
                            rhs=xb[:, k * tn : (k + 1) * tn],
                            start=(k == 0),
                            stop=(k == KD - 1),
                        )
                    for k in range(KD):
                        nc.tensor.matmul(
                            pb[:],
                            lhsT=w1b[:, k * 128 : (k + 1) * 128],
                            rhs=xb[:, k * tn : (k + 1) * tn],
                            start=(k == 0),
                            stop=(k == KD - 1),
                        )
                    sa = sap.tile([128, tn], f32, tag="sa")
                    nc.scalar.activation(sa[:], pa[:], AF.Silu, bias=bia, scale=1.0)
                    nc.vector.scalar_tensor_tensor(
                        out=gt[:, fp * L + toff : fp * L + toff + tn],
                        in0=pb[:],
                        scalar=bib,
                        in1=sa[:],
                        op0=ALU.add,
                        op1=ALU.mult,
                    )

            # GEMM2 partial: yT[d 128, tok] = sum_{k<KHS} W2s[h_k,d]^T g[h_k,tok]
            last_e = e == E - 1
            g2tiles = _chunks(L)

            def _g2(pt, toff, tn, dh):
                for k in range(KHS):
                    nc.tensor.matmul(
                        pt[:],
                        lhsT=w2sb[:, k * D + dh * 128 : k * D + (dh + 1) * 128],
                        rhs=gt[:, k * L + toff : k * L + toff + tn],
                        start=(k == 0),
                        stop=(k == KHS - 1),
                    )

            for ci, (toff, tn) in enumerate(g2tiles):
                last_c = last_e and ci == len(g2tiles) - 1
                for dh in range(DT):
                    ydst = y[dh * 128 : (dh + 1) * 128, yoff + toff : yoff + toff + tn]
                    if last_c and dh == DT - 1:
                        # Final tile: two unequal accumulation groups so only
                        # a 128-col copy + DMA remain after the last matmul.
                        h = tn - min(128, max(8, (tn // 2 // 8) * 8))
                        ptA = p2.tile([128, h], f32, tag="p2")
                        _g2(ptA, toff, h, dh)
                        ysbA = yp.tile([128, h], bf16, tag="y")
                        nc.vector.tensor_scalar_mul(ysbA[:], ptA[:], 1.0)
                        nc.gpsimd.dma_start(ydst[:, :h], ysbA[:])
                        ptB = p2.tile([128, tn - h], f32, tag="p2")
                        _g2(ptB, toff + h, tn - h, dh)
                        ysbB = yp.tile([128, tn - h], bf16, tag="y")
                        nc.scalar.copy(ysbB[:], ptB[:])
                        nc.sync.dma_start(ydst[:, h:], ysbB[:])
                    else:
                        pt = p2.tile([128, tn], f32, tag="p2")
                        _g2(pt, toff, tn, dh)
                        ysb = yp.tile([128, tn], bf16, tag="y")
                        if ndrain % 2 == 0:
                            nc.scalar.copy(ysb[:], pt[:])
                        else:
                            nc.vector.tensor_scalar_mul(ysb[:], pt[:], 1.0)
                        if last_c and dh == DT - 2:
                            nc.sync.dma_start(ydst, ysb[:])
                        elif ndrain % 2 == 0:
                            nc.gpsimd.dma_start(ydst, ysb[:])
                        else:
                            nc.sync.dma_start(ydst, ysb[:])
                        ndrain += 1
            yoff += L
    nc.compile()
    return nc


def _get_nc(Ls):
    nc = _NC_CACHE.get(Ls)
    if nc is None:
        nc = _NC_CACHE[Ls] = _build_nc(Ls)
    return nc


def _reorder_weights(W1, W2, b1):
    key = (W1.__array_interface__["data"][0], W2.__array_interface__["data"][0])
    hit = _WCACHE.get(key)
    if hit is not None:
        return hit
    import ml_dtypes

    W1 = np.ascontiguousarray(W1, dtype=np.float32)
    W2 = np.ascontiguousarray(W2, dtype=np.float32)
    b1 = np.ascontiguousarray(b1, dtype=np.float32)
    # W1 [E, D, 2H] -> [E, FP, 128p(d within k), W1T]
    w1f = (
        W1.reshape(E, KD, 128, FB, 128)
        .transpose(0, 3, 2, 1, 4)
        .reshape(E, FB, 128, KD * 128)
        .astype(ml_dtypes.bfloat16)
    )
    b1a = b1[:, :H].reshape(E, FP, 128, 1)
    b1b = b1[:, H:].reshape(E, FP, 128, 1)
    baug = np.ascontiguousarray(np.concatenate([b1a, b1b], axis=-1)).view(
        ml_dtypes.bfloat16
    )  # [E, FP, 128, 4]
    w1r = np.ascontiguousarray(
        np.concatenate([w1f[:, :FP], w1f[:, FP:], baug], axis=-1)
    )  # [E, FP, 128, W1T]
    # W2 [E, H, D] -> [E, 128p(h within k), KH*D]
    w2t = np.ascontiguousarray(
        W2.reshape(E, H // 128, 128, D)
        .transpose(0, 2, 1, 3)
        .reshape(E, 128, (H // 128) * D)
        .astype(ml_dtypes.bfloat16)
    )
    out = (w1r, w2t)
    _WCACHE.clear()
    _WCACHE[key] = out
    return out


def _route(x_flat, Wr):
    logits = x_flat @ np.ascontiguousarray(Wr, dtype=np.float32)  # [N, E]
    lmax = logits.max(axis=-1, keepdims=True)
    p = np.exp(logits - lmax)
    gates = p / p.sum(axis=-1, keepdims=True)
    expert = np.argmax(gates, axis=-1)
    order = np.argsort(expert, kind="stable")
    sorted_e = expert[order]
    starts = np.searchsorted(sorted_e, np.arange(E))
    within = np.arange(N) - starts[sorted_e]
    slot = np.empty(N, np.int64)
    slot[order] = within
    kept = slot < C
    top_idx = np.zeros((C, E), np.int32)
    valid = np.zeros((C, E), np.float32)
    tok = np.arange(N, dtype=np.int32)
    top_idx[slot[kept], expert[kept]] = tok[kept]
    valid[slot[kept], expert[kept]] = 1.0
    w_ce = gates[top_idx, np.arange(E)[None, :]].astype(np.float32) * valid  # [C, E]
    n_kept = np.minimum(np.bincount(expert, minlength=E), C)  # [E]
    return gates, expert, kept, top_idx, valid, w_ce, n_kept


def kernel(x, Wr, W1, b1, W2, b2, W1f, b1f, W2f, b2f, _trace=False):
    global LAST
    _ensure_concourse()
    import ml_dtypes
    from concourse.bass_utils import run_bass_kernel_spmd

    x_flat = np.ascontiguousarray(np.asarray(x).reshape(N, D), dtype=np.float32)
    gates, expert, kept, top_idx, valid, w_ce, n_kept = _route(x_flat, np.asarray(Wr))
    w1r, w2t = _reorder_weights(np.asarray(W1), np.asarray(W2), np.asarray(b1))

    # Experts ordered largest-first so the program tail lands on the
    # smallest expert (shortest final drain).
    eorder = [int(i) for i in np.argsort(-n_kept, kind="stable")]
    Ls = tuple(max(8, -(-int(n_kept[e]) // 8) * 8) for e in eorder)

    nc = _get_nc(Ls)

    # x blocks are identical for every core: build once.
    xparts = []
    for s, e in enumerate(eorder):
        ids = top_idx[: n_kept[e], e]
        xg = np.zeros((Ls[s], D), np.float32)
        xg[: len(ids)] = x_flat[ids]
        for c0, cn in _g1_chunks(Ls[s], s == 0):
            xparts.append(
                xg[c0 : c0 + cn]
                .reshape(cn, KD, 128)
                .transpose(2, 1, 0)
                .reshape(128, KD * cn)
                .astype(ml_dtypes.bfloat16)
            )
    xall = np.concatenate(xparts, axis=1)
    xt_shared = np.ascontiguousarray(xall[:, KD * _g1_chunks(Ls[0], True)[0][1] :])
    pro_x_block = xparts[0]

    w1o = w1r[eorder]  # [E, FP, 128, W1T]
    w2o = w2t[eorder]  # [E, 128, KH*D]
    in_maps = []
    for c in range(NCORES):
        w1c = np.ascontiguousarray(w1o[:, c * FPS : (c + 1) * FPS])
        w2c = np.ascontiguousarray(w2o[:, :, c * KHS * D : (c + 1) * KHS * D])
        pro_c = np.ascontiguousarray(
            np.concatenate([pro_x_block, w1c[0, 0]], axis=1)
        )
        in_maps.append({"pro": pro_c, "xt": xt_shared, "w1r": w1c, "w2t": w2c})
    res = run_bass_kernel_spmd(nc, in_maps, list(range(NCORES)), trace=_trace)
    LAST = res

    # Combine: sum the 8 partial yT, gate-weight + scatter to token order.
    ysum = res.results[0]["y"].astype(np.float32)
    for c in range(1, NCORES):
        ysum += res.results[c]["y"].astype(np.float32)
    y_flat = np.zeros((N, D), np.float32)
    b2 = np.asarray(b2)
    add_b2 = bool(np.any(b2))
    off = 0
    for s, e in enumerate(eorder):
        n = int(n_kept[e])
        ids = top_idx[:n, e]
        w = w_ce[:n, e][:, None]
        y_flat[ids] = w * ysum[:, off : off + n].T
        if add_b2:
            y_flat[ids] += w * b2[e]
        off += Ls[s]

    # Dense fallback for fully-dropped tokens (rare; none at typical loads).
    dropped = ~kept
    if np.any(dropped):
        xd = x_flat[dropped]
        hf = xd @ np.asarray(W1f) + np.asarray(b1f)
        gf = (hf[:, :H] / (1.0 + np.exp(-hf[:, :H]))) * hf[:, H:]
        y_flat[dropped] += FALLBACK_W * (gf @ np.asarray(W2f) + np.asarray(b2f))

    return y_flat.reshape(B, S, D)


# revision 21
# speedup vs baseline: 1.2953x; 1.2953x over previous
"""MoE feed-forward (top-1 routing, capacity 640, swiglu experts) on 8 trn2 cores.

Strategy (expert-parallel, per the sharding hint):
  * Host: router matmul/softmax/argmax + capacity-slot assignment (index
    plumbing, ~0.1% of FLOPs), gathers tokens per expert, pairs a heavy
    expert with a light one per core (greedy balance), 2 experts per core.
  * Device (Bass/Tile, per core): grouped GEMM  h = x @ W1  -> swiglu ->
    yT = W2^T @ g, in bf16 with fp32 accumulate.  Both GEMMs keep tokens in
    the moving (free) dimension, so token counts are exact (rounded to 8)
    rather than padded to 128: GEMM1 produces hT [feat, tok], GEMM2
    produces yT [d, tok].  Combine-gate scaling and the scatter back to
    token order happen on the host, so no on-chip transpose is needed.
  * Startup: each dma_start costs ~1us serialized (descriptor-gen +
    doorbell), so the critical first data (x token-chunk 0 + W1 tile 0 +
    its biases) is packed into ONE prologue DMA; b1 biases ride inside
    every W1 tile (bitcast bf16 pairs) so no tiny-descriptor bias DMA jams
    the queues; dummy matmuls on a zeroed tile warm the PE clock gate
    (HAM un-throttle) during the initial DMA wait.
  * DMA pacing: W2 is streamed in per-k chunks interleaved with the W1
    tile stream on the same (sync) queue so the bulk W2 load cannot starve
    the W1 tiles GEMM1 is consuming.
  * Tail: the very last GEMM2 accumulation is split into two half-width
    PSUM groups drained on different engines/queues to shorten the
    end-of-kernel chain.
  * Host: scatter weighted expert outputs back to token order; dense
    fallback FFN applied only to dropped tokens (none at typical loads).
"""

import os
import sys

import numpy as np


def _ensure_concourse():
    try:
        import concourse.bass  # noqa: F401
    except Exception:
        for p in ("/opt/trn_rl_repo", "/root/.axon_site/_ro/trn_rl_repo"):
            if os.path.isdir(p) and p not in sys.path:
                sys.path.insert(0, p)
        import concourse.bass  # noqa: F401


# Problem constants (hardcoded per the task contract).
B, S, D, H, E = 4, 2048, 768, 3072, 16
N = B * S
C = 640  # capacity per expert (ceil(1.25 * N / E))
FALLBACK_W = 1.0
NCORES = 8
EL = E // NCORES  # experts per core = 2
KD = D // 128  # 6 k-tiles for GEMM1 contraction
FB = (2 * H) // 128  # 48 feature blocks of GEMM1 output
FP = FB // 2  # 24 swiglu pairs == k-tiles of GEMM2 contraction
KH = H // 128  # 24
DT = D // 128  # 6 output d-tiles of GEMM2
W1W = 2 * KD * 128  # 1536 weight columns of one W1 tile
W1T = W1W + 4  # + 2 fp32 bias columns packed as 4 bf16
NWARM = 40  # dummy matmuls to pre-warm the PE clock gate

_NC_CACHE = {}  # (L0, L1) -> compiled Bass program
_WCACHE = {}  # weight reorder cache
LAST = None  # BassKernelResults of the most recent run (for profiling)


def _chunks(L):
    """Split token count L into near-equal moving chunks <= 512 (8-aligned)."""
    nch = -(-L // 512)
    base = -(-(-(-L // nch)) // 8) * 8
    out, off = [], 0
    for _ in range(nch - 1):
        out.append((off, base))
        off += base
    out.append((off, L - off))
    return out


def _g1_chunks(L):
    """GEMM1 chunking: a small 128-token head chunk (expert 0's rides the
    prologue DMA); an n-col + (L-n)-col matmul pair costs exactly what two
    L/2-col matmuls do, so this is free."""
    if L <= 512:
        return [(0, L)]
    return [(0, 128), (128, L - 128)]


def _build_nc(Ls):
    """Per-core Bass program: 2 expert slots with Ls[s] (8-aligned) tokens."""
    import concourse.bacc as bacc
    import concourse.mybir as mybir
    import concourse.tile as tile
    from contextlib import ExitStack

    f32 = mybir.dt.float32
    bf16 = mybir.dt.bfloat16
    AF = mybir.ActivationFunctionType
    ALU = mybir.AluOpType

    L0, L1 = Ls
    tot = L0 + L1
    g1t0 = _g1_chunks(L0)
    pro_x = KD * g1t0[0][1]  # x columns in the prologue pack

    nc = bacc.Bacc("TRN2", target_bir_lowering=False)
    # Host-side layouts are pre-tiled so every DMA is 2D [128, contiguous].
    # pro packs expert-0's first x token-chunk + W1 tile 0 (incl. biases).
    # xt is chunk-major: for each expert slot, for each remaining token
    # chunk (c0, cn), a [128, KD * cn] block.
    pro = nc.dram_tensor("pro", [128, pro_x + W1T], bf16, kind="ExternalInput")
    xt = nc.dram_tensor("xt", [128, KD * tot - pro_x], bf16, kind="ExternalInput")
    w1r = nc.dram_tensor("w1r", [EL, FP, 128, W1T], bf16, kind="ExternalInput")
    w2t = nc.dram_tensor("w2t", [EL, 128, KH * D], bf16, kind="ExternalInput")
    y = nc.dram_tensor("y", [D, tot], bf16, kind="ExternalOutput")

    with tile.TileContext(nc) as tc, ExitStack() as ctx:
        prop = ctx.enter_context(tc.tile_pool(name="prop", bufs=1))
        xp = ctx.enter_context(tc.tile_pool(name="xp", bufs=2))
        w2p = ctx.enter_context(tc.tile_pool(name="w2p", bufs=2))
        gp = ctx.enter_context(tc.tile_pool(name="gp", bufs=2))
        w1p = ctx.enter_context(tc.tile_pool(name="w1p", bufs=6))
        sap = ctx.enter_context(tc.tile_pool(name="sap", bufs=3))
        cst = ctx.enter_context(tc.tile_pool(name="cst", bufs=1))
        yp = ctx.enter_context(tc.tile_pool(name="yp", bufs=4))
        p1 = ctx.enter_context(tc.tile_pool(name="p1", bufs=3, space="PSUM"))
        p2 = ctx.enter_context(tc.tile_pool(name="p2", bufs=2, space="PSUM"))

        # Prologue pack: first load on the sync queue.
        pro_sb = prop.tile([128, pro_x + W1T], bf16, tag="pro")
        nc.sync.dma_start(pro_sb[:], pro[:])

        # Pre-warm the PE clock gate (HAM) with dummy matmuls on a zeroed
        # tile while the first DMAs are in flight: ~2.8us of PE activity
        # un-throttles the clock to 2.4 GHz as the real stream begins.
        zt = cst.tile([128, 128], bf16, tag="zt")
        nc.vector.memset(zt[:], 0.0)
        pz = p2.tile([128, 128], f32, tag="p2")
        for i in range(NWARM):
            nc.tensor.matmul(
                pz[:], lhsT=zt[:], rhs=zt[:], start=(i == 0), stop=(i == NWARM - 1)
            )

        xoff = 0  # running column offset into xt
        for e in range(EL):
            L = Ls[e]
            yoff = L0 if e else 0
            tiles = _g1_chunks(L)
            # x blocks: expert 0's first chunk lives in the prologue pack;
            # everything else is a [128, KD*cn] chunk-major block in xt.
            xaps = []
            rest = [t for t in tiles]
            if e == 0:
                xaps.append(pro_sb[:, :pro_x])
                rest = tiles[1:]
            if rest:
                xw = sum(KD * cn for _, cn in rest)
                xsb = xp.tile([128, xw], bf16, tag="x")
                boff = 0
                for _, cn in rest:
                    xaps.append(xsb[:, boff : boff + KD * cn])
                    nc.sync.dma_start(
                        xsb[:, boff : boff + KD * cn],
                        xt[:, xoff : xoff + KD * cn],
                    )
                    boff += KD * cn
                    xoff += KD * cn

            gt = gp.tile([128, KH * L], bf16, tag="g")
            w2sb = w2p.tile([128, KH * D], bf16, tag="w2")

            # GEMM1 + swiglu: hT tiles [feat 128, tok chunk]
            for fp in range(FP):
                if e == 0 and fp == 0:
                    w1t = pro_sb[:, pro_x:]
                else:
                    w1t = w1p.tile([128, W1T], bf16, tag="w1")
                    nc.sync.dma_start(w1t[:], w1r[e, fp, :, :])
                    w1t = w1t[:]
                # Trickle W2 through the same queue (2 chunks per fp from
                # fp=8) so it is resident before GEMM2 without ever
                # bursting ahead of the W1 stream.
                if 8 <= fp < 20:
                    j = 2 * (fp - 8)
                    nc.sync.dma_start(
                        w2sb[:, j * D : (j + 2) * D], w2t[e, :, j * D : (j + 2) * D]
                    )
                w1a = w1t[:, : KD * 128]
                w1b = w1t[:, KD * 128 : W1W]
                bia = w1t[:, W1W : W1W + 2].bitcast(f32)
                bib = w1t[:, W1W + 2 : W1W + 4].bitcast(f32)
                for ci, (toff, tn) in enumerate(tiles):
                    xb = xaps[ci]
                    pa = p1.tile([128, tn], f32, tag="pa")
                    pb = p1.tile([128, tn], f32, tag="pb")
                    for k in range(KD):
                        nc.tensor.matmul(
                            pa[:],
                            lhsT=w1a[:, k * 128 : (k + 1) * 128],
                            rhs=xb[:, k * tn : (k + 1) * tn],
                            start=(k == 0),
                            stop=(k == KD - 1),
                        )
                    for k in range(KD):
                        nc.tensor.matmul(
                            pb[:],
                            lhsT=w1b[:, k * 128 : (k + 1) * 128],
                            rhs=xb[:, k * tn : (k + 1) * tn],
                            start=(k == 0),
                            stop=(k == KD - 1),
                        )
                    sa = sap.tile([128, tn], f32, tag="sa")
                    # silu(a + b1_a)
                    nc.scalar.activation(sa[:], pa[:], AF.Silu, bias=bia, scale=1.0)
                    # g = (b + b1_b) * silu(...)
                    nc.vector.scalar_tensor_tensor(
                        out=gt[:, fp * L + toff : fp * L + toff + tn],
                        in0=pb[:],
                        scalar=bib,
                        in1=sa[:],
                        op0=ALU.add,
                        op1=ALU.mult,
                    )

            # GEMM2: yT[d 128, tok chunk] = sum_k W2[h_k, d]^T @ g[h_k, tok]
            # Tokens stay in the moving dim => no padding to 128 rows, and
            # the combine-gate scaling moves to the host scatter.
            last_e = e == EL - 1
            g2tiles = _chunks(L)

            def _g2(pt, toff, tn, dh):
                for k in range(KH):
                    nc.tensor.matmul(
                        pt[:],
                        lhsT=w2sb[:, k * D + dh * 128 : k * D + (dh + 1) * 128],
                        rhs=gt[:, k * L + toff : k * L + toff + tn],
                        start=(k == 0),
                        stop=(k == KH - 1),
                    )

            for ci, (toff, tn) in enumerate(g2tiles):
                last_c = last_e and ci == len(g2tiles) - 1
                for dh in range(DT):
                    ydst = y[dh * 128 : (dh + 1) * 128, yoff + toff : yoff + toff + tn]
                    if last_c and dh == DT - 1:
                        # Final tile: two half-width accumulation groups so
                        # the drain of the first half overlaps the matmuls
                        # of the second, and only a half-width copy + DMA
                        # remain after the very last matmul.
                        h = tn - min(128, max(8, (tn // 2 // 8) * 8))
                        ptA = p2.tile([128, h], f32, tag="p2")
                        _g2(ptA, toff, h, dh)
                        ysbA = yp.tile([128, h], bf16, tag="y")
                        nc.vector.tensor_scalar_mul(ysbA[:], ptA[:], 1.0)
                        nc.gpsimd.dma_start(ydst[:, :h], ysbA[:])
                        ptB = p2.tile([128, tn - h], f32, tag="p2")
                        _g2(ptB, toff + h, tn - h, dh)
                        ysbB = yp.tile([128, tn - h], bf16, tag="y")
                        nc.scalar.copy(ysbB[:], ptB[:])
                        nc.sync.dma_start(ydst[:, h:], ysbB[:])
                    else:
                        pt = p2.tile([128, tn], f32, tag="p2")
                        _g2(pt, toff, tn, dh)
                        ysb = yp.tile([128, tn], bf16, tag="y")
                        if dh % 2 == 0:
                            nc.scalar.copy(ysb[:], pt[:])
                        else:
                            nc.vector.tensor_scalar_mul(ysb[:], pt[:], 1.0)
                        if last_c and dh == DT - 2:
                            # keep the gpsimd queue empty near the end
                            nc.sync.dma_start(ydst, ysb[:])
                        else:
                            nc.gpsimd.dma_start(ydst, ysb[:])
    nc.compile()
    return nc


def _get_nc(Ls):
    nc = _NC_CACHE.get(Ls)
    if nc is None:
        nc = _NC_CACHE[Ls] = _build_nc(Ls)
    return nc


def _reorder_weights(W1, W2, b1):
    key = (W1.__array_interface__["data"][0], W2.__array_interface__["data"][0])
    hit = _WCACHE.get(key)
    if hit is not None:
        return hit
    import ml_dtypes

    W1 = np.ascontiguousarray(W1, dtype=np.float32)
    W2 = np.ascontiguousarray(W2, dtype=np.float32)
    b1 = np.ascontiguousarray(b1, dtype=np.float32)
    # W1 [E, D, 2H] -> [E, FB, 128p(d within k), KD*128(f)]
    w1f = (
        W1.reshape(E, KD, 128, FB, 128)
        .transpose(0, 3, 2, 1, 4)
        .reshape(E, FB, 128, KD * 128)
        .astype(ml_dtypes.bfloat16)
    )
    # swiglu pair (fp, fp+FP) in one block per DMA + b1 pair packed as
    # 2 fp32 (= 4 bf16) trailing columns
    b1a = b1[:, :H].reshape(E, FP, 128, 1)
    b1b = b1[:, H:].reshape(E, FP, 128, 1)
    baug = np.ascontiguousarray(np.concatenate([b1a, b1b], axis=-1)).view(
        ml_dtypes.bfloat16
    )  # [E, FP, 128, 4]
    w1r = np.ascontiguousarray(
        np.concatenate([w1f[:, :FP], w1f[:, FP:], baug], axis=-1)
    )  # [E, FP, 128, W1T]
    # W2 [E, H, D] -> [E, 128p(h within k), KH*D]
    w2t = np.ascontiguousarray(
        W2.reshape(E, KH, 128, D)
        .transpose(0, 2, 1, 3)
        .reshape(E, 128, KH * D)
        .astype(ml_dtypes.bfloat16)
    )
    out = (w1r, w2t)
    _WCACHE.clear()
    _WCACHE[key] = out
    return out


def _route(x_flat, Wr):
    logits = x_flat @ np.ascontiguousarray(Wr, dtype=np.float32)  # [N, E]
    lmax = logits.max(axis=-1, keepdims=True)
    p = np.exp(logits - lmax)
    gates = p / p.sum(axis=-1, keepdims=True)
    expert = np.argmax(gates, axis=-1)
    # slot = occurrence index of each token within its expert's queue
    order = np.argsort(expert, kind="stable")
    sorted_e = expert[order]
    starts = np.searchsorted(sorted_e, np.arange(E))
    within = np.arange(N) - starts[sorted_e]
    slot = np.empty(N, np.int64)
    slot[order] = within
    kept = slot < C
    top_idx = np.zeros((C, E), np.int32)
    valid = np.zeros((C, E), np.float32)
    tok = np.arange(N, dtype=np.int32)
    top_idx[slot[kept], expert[kept]] = tok[kept]
    valid[slot[kept], expert[kept]] = 1.0
    w_ce = gates[top_idx, np.arange(E)[None, :]].astype(np.float32) * valid  # [C, E]
    n_kept = np.minimum(np.bincount(expert, minlength=E), C)  # [E]
    return gates, expert, kept, top_idx, valid, w_ce, n_kept


def kernel(x, Wr, W1, b1, W2, b2, W1f, b1f, W2f, b2f, _trace=False):
    global LAST
    _ensure_concourse()
    import ml_dtypes
    from concourse.bass_utils import run_bass_kernel_spmd

    x_flat = np.ascontiguousarray(np.asarray(x).reshape(N, D), dtype=np.float32)
    gates, expert, kept, top_idx, valid, w_ce, n_kept = _route(x_flat, np.asarray(Wr))
    w1r, w2t = _reorder_weights(np.asarray(W1), np.asarray(W2), np.asarray(b1))

    # Pair heavy experts with light ones (greedy balance); slot 0 = heavy.
    order = np.argsort(-n_kept, kind="stable")
    assign = [(int(order[i]), int(order[E - 1 - i])) for i in range(NCORES)]
    # Slot shapes: exact max routed count per slot, rounded up to 8.
    Ls = tuple(
        max(64, -(-max(int(n_kept[a[s]]) for a in assign) // 8) * 8) for s in range(EL)
    )
    L0, L1 = Ls

    nc = _get_nc(Ls)
    in_maps = []
    for c in range(NCORES):
        exps = assign[c]
        # gather + transpose tokens for each slot, chunk-major: for each
        # token chunk (c0, cn) a [128, KD*cn] block
        xparts = []
        for s, e in enumerate(exps):
            ids = top_idx[: n_kept[e], e]
            xg = np.zeros((Ls[s], D), np.float32)
            xg[: len(ids)] = x_flat[ids]
            for c0, cn in _g1_chunks(Ls[s]):
                xparts.append(
                    xg[c0 : c0 + cn]
                    .reshape(cn, KD, 128)
                    .transpose(2, 1, 0)
                    .reshape(128, KD * cn)
                    .astype(ml_dtypes.bfloat16)
                )
        pro_c = np.ascontiguousarray(
            np.concatenate([xparts[0], w1r[exps[0], 0]], axis=1)
        )
        xt_c = np.ascontiguousarray(np.concatenate(xparts[1:], axis=1))
        el = list(exps)
        in_maps.append(
            {
                "pro": pro_c,
                "xt": xt_c,
                "w1r": np.ascontiguousarray(w1r[el]),
                "w2t": np.ascontiguousarray(w2t[el]),
            }
        )
    res = run_bass_kernel_spmd(nc, in_maps, list(range(NCORES)), trace=_trace)
    LAST = res

    # Combine: gate-weight + scatter expert outputs back to token order.
    y_flat = np.zeros((N, D), np.float32)
    b2 = np.asarray(b2)
    add_b2 = bool(np.any(b2))
    for c in range(NCORES):
        yc = res.results[c]["y"]  # [D, L0+L1] bf16
        for s, e in enumerate(assign[c]):
            n = int(n_kept[e])
            ids = top_idx[:n, e]
            off = L0 if s else 0
            w = w_ce[:n, e][:, None]
            y_flat[ids] = w * yc[:, off : off + n].T.astype(np.float32)
            if add_b2:
                y_flat[ids] += w * b2[e]

    # Dense fallback for fully-dropped tokens (rare; none at typical loads).
    dropped = ~kept
    if np.any(dropped):
        xd = x_flat[dropped]
        hf = xd @ np.asarray(W1f) + np.asarray(b1f)
        gf = (hf[:, :H] / (1.0 + np.exp(-hf[:, :H]))) * hf[:, H:]
        y_flat[dropped] += FALLBACK_W * (gf @ np.asarray(W2f) + np.asarray(b2f))

    return y_flat.reshape(B, S, D)


# revision 22
# speedup vs baseline: 1.2972x; 1.0014x over previous
"""MoE feed-forward (top-1 routing, capacity 640, swiglu experts) on 8 trn2 cores.

Strategy (expert-parallel, per the sharding hint):
  * Host: router matmul/softmax/argmax + capacity-slot assignment (index
    plumbing, ~0.1% of FLOPs), gathers tokens per expert, pairs a heavy
    expert with a light one per core (greedy balance), 2 experts per core.
  * Device (Bass/Tile, per core): grouped GEMM  h = x @ W1  -> swiglu ->
    yT = W2^T @ g, in bf16 with fp32 accumulate.  Both GEMMs keep tokens in
    the moving (free) dimension, so token counts are exact (rounded to 8)
    rather than padded to 128: GEMM1 produces hT [feat, tok], GEMM2
    produces yT [d, tok].  Combine-gate scaling and the scatter back to
    token order happen on the host, so no on-chip transpose is needed.
  * Startup: each dma_start costs ~1us serialized (descriptor-gen +
    doorbell), so the critical first data (x token-chunk 0 + W1 tile 0 +
    its biases) is packed into ONE prologue DMA; b1 biases ride inside
    every W1 tile (bitcast bf16 pairs) so no tiny-descriptor bias DMA jams
    the queues; dummy matmuls on a zeroed tile warm the PE clock gate
    (HAM un-throttle) during the initial DMA wait.
  * DMA pacing: W2 is streamed in per-k chunks interleaved with the W1
    tile stream on the same (sync) queue so the bulk W2 load cannot starve
    the W1 tiles GEMM1 is consuming.
  * Tail: the very last GEMM2 accumulation is split into two half-width
    PSUM groups drained on different engines/queues to shorten the
    end-of-kernel chain.
  * Host: scatter weighted expert outputs back to token order; dense
    fallback FFN applied only to dropped tokens (none at typical loads).
"""

import os
import sys

import numpy as np


def _ensure_concourse():
    try:
        import concourse.bass  # noqa: F401
    except Exception:
        for p in ("/opt/trn_rl_repo", "/root/.axon_site/_ro/trn_rl_repo"):
            if os.path.isdir(p) and p not in sys.path:
                sys.path.insert(0, p)
        import concourse.bass  # noqa: F401


# Problem constants (hardcoded per the task contract).
B, S, D, H, E = 4, 2048, 768, 3072, 16
N = B * S
C = 640  # capacity per expert (ceil(1.25 * N / E))
FALLBACK_W = 1.0
NCORES = 8
EL = E // NCORES  # experts per core = 2
KD = D // 128  # 6 k-tiles for GEMM1 contraction
FB = (2 * H) // 128  # 48 feature blocks of GEMM1 output
FP = FB // 2  # 24 swiglu pairs == k-tiles of GEMM2 contraction
KH = H // 128  # 24
DT = D // 128  # 6 output d-tiles of GEMM2
W1W = 2 * KD * 128  # 1536 weight columns of one W1 tile
W1T = W1W + 4  # + 2 fp32 bias columns packed as 4 bf16
NWARM = 40  # dummy matmuls to pre-warm the PE clock gate

_NC_CACHE = {}  # (L0, L1) -> compiled Bass program
_WCACHE = {}  # weight reorder cache
LAST = None  # BassKernelResults of the most recent run (for profiling)


def _chunks(L):
    """Split token count L into near-equal moving chunks <= 512 (8-aligned)."""
    nch = -(-L // 512)
    base = -(-(-(-L // nch)) // 8) * 8
    out, off = [], 0
    for _ in range(nch - 1):
        out.append((off, base))
        off += base
    out.append((off, L - off))
    return out


def _g1_chunks(L):
    """GEMM1 chunking: a small 128-token head chunk (expert 0's rides the
    prologue DMA); an n-col + (L-n)-col matmul pair costs exactly what two
    L/2-col matmuls do, so this is free."""
    if L <= 512:
        return [(0, L)]
    return [(0, 128), (128, L - 128)]


def _build_nc(Ls):
    """Per-core Bass program: 2 expert slots with Ls[s] (8-aligned) tokens."""
    import concourse.bacc as bacc
    import concourse.mybir as mybir
    import concourse.tile as tile
    from contextlib import ExitStack

    f32 = mybir.dt.float32
    bf16 = mybir.dt.bfloat16
    AF = mybir.ActivationFunctionType
    ALU = mybir.AluOpType

    L0, L1 = Ls
    tot = L0 + L1
    g1t0 = _g1_chunks(L0)
    pro_x = KD * g1t0[0][1]  # x columns in the prologue pack

    nc = bacc.Bacc("TRN2", target_bir_lowering=False)
    # Host-side layouts are pre-tiled so every DMA is 2D [128, contiguous].
    # pro packs expert-0's first x token-chunk + W1 tile 0 (incl. biases).
    # xt is chunk-major: for each expert slot, for each remaining token
    # chunk (c0, cn), a [128, KD * cn] block.
    pro = nc.dram_tensor("pro", [128, pro_x + W1T], bf16, kind="ExternalInput")
    xt = nc.dram_tensor("xt", [128, KD * tot - pro_x], bf16, kind="ExternalInput")
    w1r = nc.dram_tensor("w1r", [EL, FP, 128, W1T], bf16, kind="ExternalInput")
    w2t = nc.dram_tensor("w2t", [EL, 128, KH * D], bf16, kind="ExternalInput")
    y = nc.dram_tensor("y", [D, tot], bf16, kind="ExternalOutput")

    with tile.TileContext(nc) as tc, ExitStack() as ctx:
        prop = ctx.enter_context(tc.tile_pool(name="prop", bufs=1))
        xp = ctx.enter_context(tc.tile_pool(name="xp", bufs=2))
        w2p = ctx.enter_context(tc.tile_pool(name="w2p", bufs=2))
        gp = ctx.enter_context(tc.tile_pool(name="gp", bufs=2))
        w1p = ctx.enter_context(tc.tile_pool(name="w1p", bufs=8))
        sap = ctx.enter_context(tc.tile_pool(name="sap", bufs=3))
        cst = ctx.enter_context(tc.tile_pool(name="cst", bufs=1))
        yp = ctx.enter_context(tc.tile_pool(name="yp", bufs=4))
        p1 = ctx.enter_context(tc.tile_pool(name="p1", bufs=3, space="PSUM"))
        p2 = ctx.enter_context(tc.tile_pool(name="p2", bufs=2, space="PSUM"))

        # Prologue pack: first load on the sync queue.
        pro_sb = prop.tile([128, pro_x + W1T], bf16, tag="pro")
        nc.sync.dma_start(pro_sb[:], pro[:])

        # Pre-warm the PE clock gate (HAM) with dummy matmuls on a zeroed
        # tile while the first DMAs are in flight: ~2.8us of PE activity
        # un-throttles the clock to 2.4 GHz as the real stream begins.
        zt = cst.tile([128, 128], bf16, tag="zt")
        nc.vector.memset(zt[:], 0.0)
        pz = p2.tile([128, 128], f32, tag="p2")
        for i in range(NWARM):
            nc.tensor.matmul(
                pz[:], lhsT=zt[:], rhs=zt[:], start=(i == 0), stop=(i == NWARM - 1)
            )

        xoff = 0  # running column offset into xt
        for e in range(EL):
            L = Ls[e]
            yoff = L0 if e else 0
            tiles = _g1_chunks(L)
            # x blocks: expert 0's first chunk lives in the prologue pack;
            # everything else is a [128, KD*cn] chunk-major block in xt.
            xaps = []
            rest = [t for t in tiles]
            if e == 0:
                xaps.append(pro_sb[:, :pro_x])
                rest = tiles[1:]
            if rest:
                xw = sum(KD * cn for _, cn in rest)
                xsb = xp.tile([128, xw], bf16, tag="x")
                boff = 0
                for _, cn in rest:
                    xaps.append(xsb[:, boff : boff + KD * cn])
                    nc.sync.dma_start(
                        xsb[:, boff : boff + KD * cn],
                        xt[:, xoff : xoff + KD * cn],
                    )
                    boff += KD * cn
                    xoff += KD * cn

            gt = gp.tile([128, KH * L], bf16, tag="g")
            w2sb = w2p.tile([128, KH * D], bf16, tag="w2")

            # GEMM1 + swiglu: hT tiles [feat 128, tok chunk]
            for fp in range(FP):
                if e == 0 and fp == 0:
                    w1t = pro_sb[:, pro_x:]
                else:
                    w1t = w1p.tile([128, W1T], bf16, tag="w1")
                    nc.sync.dma_start(w1t[:], w1r[e, fp, :, :])
                    w1t = w1t[:]
                # Trickle W2 through the same queue (2 chunks per fp from
                # fp=8) so it is resident before GEMM2 without ever
                # bursting ahead of the W1 stream.
                if 8 <= fp < 20:
                    j = 2 * (fp - 8)
                    nc.sync.dma_start(
                        w2sb[:, j * D : (j + 2) * D], w2t[e, :, j * D : (j + 2) * D]
                    )
                w1a = w1t[:, : KD * 128]
                w1b = w1t[:, KD * 128 : W1W]
                bia = w1t[:, W1W : W1W + 2].bitcast(f32)
                bib = w1t[:, W1W + 2 : W1W + 4].bitcast(f32)
                for ci, (toff, tn) in enumerate(tiles):
                    xb = xaps[ci]
                    pa = p1.tile([128, tn], f32, tag="pa")
                    pb = p1.tile([128, tn], f32, tag="pb")
                    for k in range(KD):
                        nc.tensor.matmul(
                            pa[:],
                            lhsT=w1a[:, k * 128 : (k + 1) * 128],
                            rhs=xb[:, k * tn : (k + 1) * tn],
                            start=(k == 0),
                            stop=(k == KD - 1),
                        )
                    for k in range(KD):
                        nc.tensor.matmul(
                            pb[:],
                            lhsT=w1b[:, k * 128 : (k + 1) * 128],
                            rhs=xb[:, k * tn : (k + 1) * tn],
                            start=(k == 0),
                            stop=(k == KD - 1),
                        )
                    sa = sap.tile([128, tn], f32, tag="sa")
                    # silu(a + b1_a)
                    nc.scalar.activation(sa[:], pa[:], AF.Silu, bias=bia, scale=1.0)
                    # g = (b + b1_b) * silu(...)
                    nc.vector.scalar_tensor_tensor(
                        out=gt[:, fp * L + toff : fp * L + toff + tn],
                        in0=pb[:],
                        scalar=bib,
                        in1=sa[:],
                        op0=ALU.add,
                        op1=ALU.mult,
                    )

            # GEMM2: yT[d 128, tok chunk] = sum_k W2[h_k, d]^T @ g[h_k, tok]
            # Tokens stay in the moving dim => no padding to 128 rows, and
            # the combine-gate scaling moves to the host scatter.
            last_e = e == EL - 1
            g2tiles = _chunks(L)

            def _g2(pt, toff, tn, dh):
                for k in range(KH):
                    nc.tensor.matmul(
                        pt[:],
                        lhsT=w2sb[:, k * D + dh * 128 : k * D + (dh + 1) * 128],
                        rhs=gt[:, k * L + toff : k * L + toff + tn],
                        start=(k == 0),
                        stop=(k == KH - 1),
                    )

            for ci, (toff, tn) in enumerate(g2tiles):
                last_c = last_e and ci == len(g2tiles) - 1
                for dh in range(DT):
                    ydst = y[dh * 128 : (dh + 1) * 128, yoff + toff : yoff + toff + tn]
                    if last_c and dh == DT - 1:
                        # Final tile: two half-width accumulation groups so
                        # the drain of the first half overlaps the matmuls
                        # of the second, and only a half-width copy + DMA
                        # remain after the very last matmul.
                        h = tn - min(64, max(8, (tn // 2 // 8) * 8))
                        ptA = p2.tile([128, h], f32, tag="p2")
                        _g2(ptA, toff, h, dh)
                        ysbA = yp.tile([128, h], bf16, tag="y")
                        nc.vector.tensor_scalar_mul(ysbA[:], ptA[:], 1.0)
                        nc.gpsimd.dma_start(ydst[:, :h], ysbA[:])
                        ptB = p2.tile([128, tn - h], f32, tag="p2")
                        _g2(ptB, toff + h, tn - h, dh)
                        ysbB = yp.tile([128, tn - h], bf16, tag="y")
                        nc.scalar.copy(ysbB[:], ptB[:])
                        nc.sync.dma_start(ydst[:, h:], ysbB[:])
                    else:
                        pt = p2.tile([128, tn], f32, tag="p2")
                        _g2(pt, toff, tn, dh)
                        ysb = yp.tile([128, tn], bf16, tag="y")
                        if dh % 2 == 0:
                            nc.scalar.copy(ysb[:], pt[:])
                        else:
                            nc.vector.tensor_scalar_mul(ysb[:], pt[:], 1.0)
                        if last_c and dh == DT - 2:
                            # keep the gpsimd queue empty near the end
                            nc.sync.dma_start(ydst, ysb[:])
                        else:
                            nc.gpsimd.dma_start(ydst, ysb[:])
    nc.compile()
    return nc


def _get_nc(Ls):
    nc = _NC_CACHE.get(Ls)
    if nc is None:
        nc = _NC_CACHE[Ls] = _build_nc(Ls)
    return nc


def _reorder_weights(W1, W2, b1):
    key = (W1.__array_interface__["data"][0], W2.__array_interface__["data"][0])
    hit = _WCACHE.get(key)
    if hit is not None:
        return hit
    import ml_dtypes

    W1 = np.ascontiguousarray(W1, dtype=np.float32)
    W2 = np.ascontiguousarray(W2, dtype=np.float32)
    b1 = np.ascontiguousarray(b1, dtype=np.float32)
    # W1 [E, D, 2H] -> [E, FB, 128p(d within k), KD*128(f)]
    w1f = (
        W1.reshape(E, KD, 128, FB, 128)
        .transpose(0, 3, 2, 1, 4)
        .reshape(E, FB, 128, KD * 128)
        .astype(ml_dtypes.bfloat16)
    )
    # swiglu pair (fp, fp+FP) in one block per DMA + b1 pair packed as
    # 2 fp32 (= 4 bf16) trailing columns
    b1a = b1[:, :H].reshape(E, FP, 128, 1)
    b1b = b1[:, H:].reshape(E, FP, 128, 1)
    baug = np.ascontiguousarray(np.concatenate([b1a, b1b], axis=-1)).view(
        ml_dtypes.bfloat16
    )  # [E, FP, 128, 4]
    w1r = np.ascontiguousarray(
        np.concatenate([w1f[:, :FP], w1f[:, FP:], baug], axis=-1)
    )  # [E, FP, 128, W1T]
    # W2 [E, H, D] -> [E, 128p(h within k), KH*D]
    w2t = np.ascontiguousarray(
        W2.reshape(E, KH, 128, D)
        .transpose(0, 2, 1, 3)
        .reshape(E, 128, KH * D)
        .astype(ml_dtypes.bfloat16)
    )
    out = (w1r, w2t)
    _WCACHE.clear()
    _WCACHE[key] = out
    return out


def _route(x_flat, Wr):
    logits = x_flat @ np.ascontiguousarray(Wr, dtype=np.float32)  # [N, E]
    lmax = logits.max(axis=-1, keepdims=True)
    p = np.exp(logits - lmax)
    gates = p / p.sum(axis=-1, keepdims=True)
    expert = np.argmax(gates, axis=-1)
    # slot = occurrence index of each token within its expert's queue
    order = np.argsort(expert, kind="stable")
    sorted_e = expert[order]
    starts = np.searchsorted(sorted_e, np.arange(E))
    within = np.arange(N) - starts[sorted_e]
    slot = np.empty(N, np.int64)
    slot[order] = within
    kept = slot < C
    top_idx = np.zeros((C, E), np.int32)
    valid = np.zeros((C, E), np.float32)
    tok = np.arange(N, dtype=np.int32)
    top_idx[slot[kept], expert[kept]] = tok[kept]
    valid[slot[kept], expert[kept]] = 1.0
    w_ce = gates[top_idx, np.arange(E)[None, :]].astype(np.float32) * valid  # [C, E]
    n_kept = np.minimum(np.bincount(expert, minlength=E), C)  # [E]
    return gates, expert, kept, top_idx, valid, w_ce, n_kept


def kernel(x, Wr, W1, b1, W2, b2, W1f, b1f, W2f, b2f, _trace=False):
    global LAST
    _ensure_concourse()
    import ml_dtypes
    from concourse.bass_utils import run_bass_kernel_spmd

    x_flat = np.ascontiguousarray(np.asarray(x).reshape(N, D), dtype=np.float32)
    gates, expert, kept, top_idx, valid, w_ce, n_kept = _route(x_flat, np.asarray(Wr))
    w1r, w2t = _reorder_weights(np.asarray(W1), np.asarray(W2), np.asarray(b1))

    # Pair heavy experts with light ones (greedy balance); slot 0 = heavy.
    order = np.argsort(-n_kept, kind="stable")
    assign = [(int(order[i]), int(order[E - 1 - i])) for i in range(NCORES)]
    # Slot shapes: exact max routed count per slot, rounded up to 8.
    Ls = tuple(
        max(64, -(-max(int(n_kept[a[s]]) for a in assign) // 4) * 4) for s in range(EL)
    )
    L0, L1 = Ls

    nc = _get_nc(Ls)
    in_maps = []
    for c in range(NCORES):
        exps = assign[c]
        # gather + transpose tokens for each slot, chunk-major: for each
        # token chunk (c0, cn) a [128, KD*cn] block
        xparts = []
        for s, e in enumerate(exps):
            ids = top_idx[: n_kept[e], e]
            xg = np.zeros((Ls[s], D), np.float32)
            xg[: len(ids)] = x_flat[ids]
            for c0, cn in _g1_chunks(Ls[s]):
                xparts.append(
                    xg[c0 : c0 + cn]
                    .reshape(cn, KD, 128)
                    .transpose(2, 1, 0)
                    .reshape(128, KD * cn)
                    .astype(ml_dtypes.bfloat16)
                )
        pro_c = np.ascontiguousarray(
            np.concatenate([xparts[0], w1r[exps[0], 0]], axis=1)
        )
        xt_c = np.ascontiguousarray(np.concatenate(xparts[1:], axis=1))
        el = list(exps)
        in_maps.append(
            {
                "pro": pro_c,
                "xt": xt_c,
                "w1r": np.ascontiguousarray(w1r[el]),
                "w2t": np.ascontiguousarray(w2t[el]),
            }
        )
    res = run_bass_kernel_spmd(nc, in_maps, list(range(NCORES)), trace=_trace)
    LAST = res

    # Combine: gate-weight + scatter expert outputs back to token order.
    y_flat = np.zeros((N, D), np.float32)
    b2 = np.asarray(b2)
    add_b2 = bool(np.any(b2))
    for c in range(NCORES):
        yc = res.results[c]["y"]  # [D, L0+L1] bf16
        for s, e in enumerate(assign[c]):
            n = int(n_kept[e])
            ids = top_idx[:n, e]
            off = L0 if s else 0
            w = w_ce[:n, e][:, None]
            y_flat[ids] = w * yc[:, off : off + n].T.astype(np.float32)
            if add_b2:
                y_flat[ids] += w * b2[e]

    # Dense fallback for fully-dropped tokens (rare; none at typical loads).
    dropped = ~kept
    if np.any(dropped):
        xd = x_flat[dropped]
        hf = xd @ np.asarray(W1f) + np.asarray(b1f)
        gf = (hf[:, :H] / (1.0 + np.exp(-hf[:, :H]))) * hf[:, H:]
        y_flat[dropped] += FALLBACK_W * (gf @ np.asarray(W2f) + np.asarray(b2f))

    return y_flat.reshape(B, S, D)
